# revision 24
# baseline (speedup 1.0000x reference)
"""MoE gate (router) kernel for Trainium2, 8 NeuronCores.

reference:
    scores = x @ weight.T          # [N, E]
    probs  = softmax(scores, -1)
    vals, idx = top_k(probs, 8)
    vals  = vals / (vals.sum(-1, keepdims) + 1e-9)
    returns (vals [N,8] f32, idx [N,8] i32, probs [N,E] f32)

Sharding: x is split along the token dim across 8 cores (1024 tokens each);
weight is replicated.  Routing is token-local so no collectives are needed.

Per-core pipeline (tokens stay on partitions the whole way):
  - prep: W [64,4096] -> wT chunks [128d, 64e] via PE transpose (once)
  - per 128-token subtile:
      load x [128, 4096]
      PE-transpose 32 [128,128] chunks -> xT (PSUM) -> SBUF
      32 accumulating fp32 matmuls: scores[128t, 64e] += xT_c.T @ wT_c
      softmax on [128, 64]: DVE reduce_max(neg) -> ACT exp(bias=-max,
          accum_out=sum) -> DVE reciprocal -> ACT scale
      top-8:  DVE max / max_index (hw top-8 primitives), renormalize
"""

import numpy as np

import concourse.bass as bass
import concourse.mybir as mybir
import concourse.tile as tile
from concourse import masks
from concourse.bass_utils import run_bass_kernel_spmd
from concourse.vector_clock import ScopedClock

F32 = mybir.dt.float32
U32 = mybir.dt.uint32

N_CORES = 8
N, D, E, TOPK = 8192, 4096, 64, 8
NT = N // N_CORES        # tokens per core
ST = 128                 # tokens per subtile (partition dim)
NSUB = NT // ST          # subtiles per core
CH = 128                 # contraction chunk
NCH = D // CH            # chunks per dot product


MAX_WAITS = 1


def _split_waits(nc, maxw=MAX_WAITS):
    """neuronxcc CoreV3 codegen rejects an instruction carrying more than
    `maxw` sem waits ("Too many sync wait commands").  Move excess waits
    onto same-engine ENGINE_NOP instructions inserted just before the
    offending instruction — the sequencer blocks on them in program order,
    which is semantically identical."""
    n = 0
    for f in nc.m.functions:
        for bb in f.blocks:
            snapshot = list(bb.instructions)
            rewritten = []
            changed = False
            for inst in snapshot:
                si = inst.sync_info
                if si is not None and si.on_wait and len(si.on_wait) > maxw:
                    waits = list(si.on_wait)
                    excess, keep = waits[:-maxw], waits[-maxw:]
                    for i in range(0, len(excess), maxw):
                        # nop() appends a well-formed nop to the *current*
                        # block; pop it off and re-home it in front of
                        # `inst` in this block.
                        nop = nc.engines[inst.engine].nop(nofuse=True).ins
                        cur_insts = nc.cur_bb.bb.instructions
                        assert cur_insts[-1].name == nop.name
                        cur_insts.pop()
                        n += 1
                        nop.sync_info = mybir.SyncInfo(
                            on_wait=excess[i:i + maxw], on_update=[])
                        rewritten.append(nop)
                    inst.sync_info = mybir.SyncInfo(
                        on_wait=keep, on_update=si.on_update)
                    changed = True
                rewritten.append(inst)
            if changed:
                bb.instructions[:] = rewritten
    return n


def _patch_tail_drain():
    """Run the wait-splitting pass after TileContext finishes emitting
    everything (tail drain included)."""

    orig = tile.TileContext._drain_and_barrier

    def _drain_and_barrier(self, tick_clock, wait_clock):
        orig(self, tick_clock, wait_clock)
        _split_waits(self.nc)

    tile.TileContext._drain_and_barrier = _drain_and_barrier


_patch_tail_drain()


def _emit_body(nc, tc, pools, x_d, w_d, pv_d, pi_d, pr_d, first):
    (xload, xt_sb, xt_ps, sc_ps, wt_pool, w_pool, idp, probs_pool, stat,
     top8, wp_ps) = pools

    ident = idp.tile([128, 128], F32, tag="ident")
    if first:
        masks.make_identity(nc, ident[:])
        # PE clock warm-up: ~40 no-dep transposes run during the first DMA
        # waits so the HAM gate is at 8/8 when real work lands.
        for i in range(40):
            wu = xt_ps.tile([128, 128], F32, tag="xtp")
            nc.tensor.transpose(wu[:], ident[:], ident[:])

    # ---- weight prep: wT chunks [128 d, 64 e], one tile per 4-chunk
    # group so matmuls only wait on the copy they actually read ----
    w_sb = w_pool.tile([E, D], F32, tag="w", name="w_all")
    wt_sb = wt_pool.tile([128, NCH * E], F32, tag="wt", name="wt_all")
    nc.sync.dma_start(w_sb[:], w_d[:])
    for c4 in range(NCH // 4):
        wp = sc_ps.tile([128, 4 * E], F32, tag="sc", name=f"wp{c4}")
        for j in range(4):
            c = 4 * c4 + j
            nc.tensor.transpose(wp[:, j * E:(j + 1) * E],
                                w_sb[:, c * CH:(c + 1) * CH], ident[:E, :E])
        dst = wt_sb[:, 4 * c4 * E:(4 * c4 + 4) * E]
        if c4 % 2 == 0:
            nc.vector.tensor_copy(dst, wp[:])
        else:
            nc.scalar.copy(dst, wp[:])

    # ---- software-pipelined emission: transposes for subtile s are
    # emitted before the matmuls of subtile s-1, so the PE always has
    # transpose work ready while matmul inputs are still being copied ----
    xt_tiles = {}

    def emit_load(s):
        r0 = s * ST
        x_sb = xload.tile([ST, D], F32, tag="x", name=f"x{s}")
        nc.sync.dma_start(x_sb[:, :D // 2], x_d[r0:r0 + ST, :D // 2])
        nc.sync.dma_start(x_sb[:, D // 2:], x_d[r0:r0 + ST, D // 2:])
        return x_sb

    def emit_transpose4(s, x_sb, c4):
        # 4 transposed chunks share one PSUM bank -> one fat copy
        tp = xt_ps.tile([CH, 4 * ST], F32, tag="xtp")
        for j in range(4):
            c = 4 * c4 + j
            nc.tensor.transpose(tp[:, j * ST:(j + 1) * ST],
                                x_sb[:, c * CH:(c + 1) * CH],
                                ident[:])
        xt = xt_sb.tile([CH, 4 * ST], F32, tag="xt")
        if c4 % 2 == 0:
            nc.vector.tensor_copy(xt[:], tp[:])
        else:
            nc.scalar.copy(xt[:], tp[:])
        xt_tiles.setdefault(s, []).append(xt)

    def emit_mms(s, scores, tiles, c4):
        for c in range(4 * c4, 4 * c4 + 4):
            lhsT = tiles[c // 4][:, (c % 4) * ST:(c % 4 + 1) * ST]
            nc.tensor.matmul(scores[:], lhsT,
                             wt_sb[:, c * E:(c + 1) * E],
                             start=(c == 0), stop=(c == NCH - 1))

    def emit_softmax(s):
        r0 = s * ST
        scores = scores_tiles.pop(s)

        # softmax over the 64 experts (free dim)
        negmax = stat.tile([ST, 1], F32, tag="negmax")
        nc.vector.reduce_max(negmax[:], scores[:], axis=mybir.AxisListType.X,
                             negate=True)
        u = probs_pool.tile([ST, E], F32, tag="u")
        sumexp = stat.tile([ST, 1], F32, tag="sumexp")
        nc.scalar.activation(u[:], scores[:], mybir.ActivationFunctionType.Exp,
                             bias=negmax[:], scale=1.0, accum_out=sumexp[:])
        rinv = stat.tile([ST, 1], F32, tag="rinv")
        nc.vector.reciprocal(rinv[:], sumexp[:])
        probs = probs_pool.tile([ST, E], F32, tag="probs")
        nc.scalar.mul(probs[:], u[:], rinv[:])
        nc.sync.dma_start(pr_d[r0:r0 + ST, :], probs[:])

        # top-8 + renormalize
        vals8 = top8.tile([ST, TOPK], F32, tag="vals8")
        nc.vector.max(vals8[:], probs[:])
        idx8 = top8.tile([ST, TOPK], U32, tag="idx8")
        nc.vector.max_index(idx8[:], vals8[:], probs[:])
        s8 = stat.tile([ST, 1], F32, tag="s8")
        nc.vector.reduce_sum(s8[:], vals8[:], axis=mybir.AxisListType.X)
        s8e = stat.tile([ST, 1], F32, tag="s8e")
        nc.vector.tensor_scalar_add(s8e[:], s8[:], 1e-9)
        r8 = stat.tile([ST, 1], F32, tag="r8")
        nc.vector.reciprocal(r8[:], s8e[:])
        vals8n = top8.tile([ST, TOPK], F32, tag="vals8n")
        nc.vector.tensor_scalar_mul(vals8n[:], vals8[:], r8[:])
        nc.sync.dma_start(pv_d[r0:r0 + ST, :], vals8n[:])
        nc.sync.dma_start(pi_d[r0:r0 + ST, :], idx8[:])

    # Chunk-interleaved software pipeline: while subtile s-1's matmuls
    # accumulate, subtile s's transposes stream in between them.
    scores_tiles = {}
    x_tiles = {}
    for s in range(NSUB + 1):
        if s < NSUB:
            x_tiles[s] = emit_load(s)
        if s >= 1:
            scores_tiles[s - 1] = sc_ps.tile([ST, E], F32, tag="sc", name=f"scores{s-1}")
        for c4 in range(NCH // 4):
            if s < NSUB:
                emit_transpose4(s, x_tiles[s], c4)
            if s >= 1:
                emit_mms(s - 1, scores_tiles[s - 1], xt_tiles[s - 1], c4)
        if s >= 1:
            xt_tiles.pop(s - 1)
            emit_softmax(s - 1)


def _build(iters=1):
    nc = bass.Bass(trn_type="TRN2", target_bir_lowering=False, debug=False)
    x_d = nc.dram_tensor("x", [NT, D], F32, kind="ExternalInput").ap()
    w_d = nc.dram_tensor("weight", [E, D], F32, kind="ExternalInput").ap()
    pv_d = nc.dram_tensor("topk_vals", [NT, TOPK], F32,
                          kind="ExternalOutput").ap()
    pi_d = nc.dram_tensor("topk_idx", [NT, TOPK], U32,
                          kind="ExternalOutput").ap()
    pr_d = nc.dram_tensor("probs", [NT, E], F32, kind="ExternalOutput").ap()

    with tile.TileContext(nc) as tc:
        from contextlib import ExitStack
        with ExitStack() as ctx:
            pools = (
                ctx.enter_context(tc.tile_pool(name="xload", bufs=4)),
                ctx.enter_context(tc.tile_pool(name="xt_sb", bufs=10)),
                ctx.enter_context(tc.tile_pool(name="xt_ps", bufs=5,
                                               space="PSUM")),
                ctx.enter_context(tc.tile_pool(name="sc_ps", bufs=3,
                                               space="PSUM")),
                ctx.enter_context(tc.tile_pool(name="wt", bufs=1)),
                ctx.enter_context(tc.tile_pool(name="w", bufs=1)),
                ctx.enter_context(tc.tile_pool(name="ident", bufs=1)),
                ctx.enter_context(tc.tile_pool(name="probs", bufs=3)),
                ctx.enter_context(tc.tile_pool(name="stat", bufs=4)),
                ctx.enter_context(tc.tile_pool(name="top8", bufs=3)),
                ctx.enter_context(tc.tile_pool(name="wp_ps", bufs=2,
                                               space="PSUM")),
            )
            for it in range(iters):
                _emit_body(nc, tc, pools, x_d, w_d, pv_d, pi_d, pr_d,
                           first=(it == 0))
    return nc


_built = {}


def _get(iters=1):
    if iters not in _built:
        _built[iters] = _build(iters)
    return _built[iters]


def run(x, weight, iters=1):
    nc = _get(iters)
    in_maps = [
        {"x": np.ascontiguousarray(x[i * NT:(i + 1) * NT]),
         "weight": np.ascontiguousarray(weight)}
        for i in range(N_CORES)
    ]
    res = run_bass_kernel_spmd(nc, in_maps, list(range(N_CORES)))
    vals = np.concatenate([res.results[i]["topk_vals"] for i in range(N_CORES)])
    idx = np.concatenate([res.results[i]["topk_idx"] for i in range(N_CORES)])
    probs = np.concatenate([res.results[i]["probs"] for i in range(N_CORES)])
    return vals, idx.astype(np.int32), probs


def kernel(x, weight):
    x = np.asarray(x, dtype=np.float32)
    weight = np.asarray(weight, dtype=np.float32)
    return run(x, weight, iters=1)


# revision 28
# speedup vs baseline: 1.0022x; 1.0022x over previous
"""MoE gate (router) kernel for Trainium2, 8 NeuronCores.

reference:
    scores = x @ weight.T          # [N, E]
    probs  = softmax(scores, -1)
    vals, idx = top_k(probs, 8)
    vals  = vals / (vals.sum(-1, keepdims) + 1e-9)
    returns (vals [N,8] f32, idx [N,8] i32, probs [N,E] f32)

Sharding: x is split along the token dim across 8 cores (1024 tokens each);
weight is replicated.  Routing is token-local so no collectives are needed.

Per-core pipeline (tokens stay on partitions the whole way):
  - prep: W [64,4096] -> wT chunks [128d, 64e] via PE transpose (once)
  - per 128-token subtile:
      load x [128, 4096]
      PE-transpose 32 [128,128] chunks -> xT (PSUM) -> SBUF
      32 accumulating fp32 matmuls: scores[128t, 64e] += xT_c.T @ wT_c
      softmax on [128, 64]: DVE reduce_max(neg) -> ACT exp(bias=-max,
          accum_out=sum) -> DVE reciprocal -> ACT scale
      top-8:  DVE max / max_index (hw top-8 primitives), renormalize
"""

import numpy as np

import concourse.bass as bass
import concourse.mybir as mybir
import concourse.tile as tile
from concourse import masks
from concourse.bass_utils import run_bass_kernel_spmd
from concourse.vector_clock import ScopedClock

F32 = mybir.dt.float32
U32 = mybir.dt.uint32

N_CORES = 8
N, D, E, TOPK = 8192, 4096, 64, 8
NT = N // N_CORES        # tokens per core
ST = 128                 # tokens per subtile (partition dim)
NSUB = NT // ST          # subtiles per core
CH = 128                 # contraction chunk
NCH = D // CH            # chunks per dot product


MAX_WAITS = 1


def _split_waits(nc, maxw=MAX_WAITS):
    """neuronxcc CoreV3 codegen rejects an instruction carrying more than
    `maxw` sem waits ("Too many sync wait commands").  Move excess waits
    onto same-engine ENGINE_NOP instructions inserted just before the
    offending instruction — the sequencer blocks on them in program order,
    which is semantically identical."""
    n = 0
    for f in nc.m.functions:
        for bb in f.blocks:
            snapshot = list(bb.instructions)
            rewritten = []
            changed = False
            for inst in snapshot:
                si = inst.sync_info
                if si is not None and si.on_wait and len(si.on_wait) > maxw:
                    waits = list(si.on_wait)
                    excess, keep = waits[:-maxw], waits[-maxw:]
                    for i in range(0, len(excess), maxw):
                        # nop() appends a well-formed nop to the *current*
                        # block; pop it off and re-home it in front of
                        # `inst` in this block.
                        nop = nc.engines[inst.engine].nop(nofuse=True).ins
                        cur_insts = nc.cur_bb.bb.instructions
                        assert cur_insts[-1].name == nop.name
                        cur_insts.pop()
                        n += 1
                        nop.sync_info = mybir.SyncInfo(
                            on_wait=excess[i:i + maxw], on_update=[])
                        rewritten.append(nop)
                    inst.sync_info = mybir.SyncInfo(
                        on_wait=keep, on_update=si.on_update)
                    changed = True
                rewritten.append(inst)
            if changed:
                bb.instructions[:] = rewritten
    return n


def _patch_tail_drain():
    """Run the wait-splitting pass after TileContext finishes emitting
    everything (tail drain included)."""

    orig = tile.TileContext._drain_and_barrier

    def _drain_and_barrier(self, tick_clock, wait_clock):
        orig(self, tick_clock, wait_clock)
        _split_waits(self.nc)

    tile.TileContext._drain_and_barrier = _drain_and_barrier


_patch_tail_drain()


def _emit_body(nc, tc, pools, x_d, w_d, pv_d, pi_d, pr_d, first):
    (xload, xt_sb, xt_ps, sc_ps, wt_pool, w_pool, idp, probs_pool, stat,
     top8, wp_ps) = pools

    ident = idp.tile([128, 128], F32, tag="ident")
    if first:
        masks.make_identity(nc, ident[:])
        # PE clock warm-up: ~40 no-dep transposes run during the first DMA
        # waits so the HAM gate is at 8/8 when real work lands.
        for i in range(40):
            wu = xt_ps.tile([128, 128], F32, tag="xtp")
            nc.tensor.transpose(wu[:], ident[:], ident[:])

    # ---- weight prep: wT chunks [128 d, 64 e], one tile per 4-chunk
    # group so matmuls only wait on the copy they actually read ----
    w_sb = w_pool.tile([E, D], F32, tag="w", name="w_all")
    wt_sb = wt_pool.tile([128, NCH * E], F32, tag="wt", name="wt_all")
    nc.sync.dma_start(w_sb[:], w_d[:])
    for c4 in range(NCH // 4):
        wp = sc_ps.tile([128, 4 * E], F32, tag="sc", name=f"wp{c4}")
        for j in range(4):
            c = 4 * c4 + j
            nc.tensor.transpose(wp[:, j * E:(j + 1) * E],
                                w_sb[:, c * CH:(c + 1) * CH], ident[:E, :E])
        dst = wt_sb[:, 4 * c4 * E:(4 * c4 + 4) * E]
        if c4 % 2 == 0:
            nc.vector.tensor_copy(dst, wp[:])
        else:
            nc.scalar.copy(dst, wp[:])

    # ---- software-pipelined emission: transposes for subtile s are
    # emitted before the matmuls of subtile s-1, so the PE always has
    # transpose work ready while matmul inputs are still being copied ----
    xt_tiles = {}

    def emit_load(s):
        r0 = s * ST
        x_sb = xload.tile([ST, D], F32, tag="x", name=f"x{s}")
        nc.sync.dma_start(x_sb[:, :D // 2], x_d[r0:r0 + ST, :D // 2])
        nc.sync.dma_start(x_sb[:, D // 2:], x_d[r0:r0 + ST, D // 2:])
        return x_sb

    def emit_transpose4(s, x_sb, c4):
        # 4 transposed chunks share one PSUM bank -> one fat copy
        tp = xt_ps.tile([CH, 4 * ST], F32, tag="xtp")
        for j in range(4):
            c = 4 * c4 + j
            nc.tensor.transpose(tp[:, j * ST:(j + 1) * ST],
                                x_sb[:, c * CH:(c + 1) * CH],
                                ident[:])
        xt = xt_sb.tile([CH, 4 * ST], F32, tag="xt")
        if c4 % 2 == 0:
            nc.vector.tensor_copy(xt[:], tp[:])
        else:
            nc.scalar.copy(xt[:], tp[:])
        xt_tiles.setdefault(s, []).append(xt)

    def emit_mms(s, scores, tiles, c4):
        for c in range(4 * c4, 4 * c4 + 4):
            lhsT = tiles[c // 4][:, (c % 4) * ST:(c % 4 + 1) * ST]
            nc.tensor.matmul(scores[:], lhsT,
                             wt_sb[:, c * E:(c + 1) * E],
                             start=(c == 0), stop=(c == NCH - 1))

    def emit_softmax(s):
        r0 = s * ST
        scores = scores_tiles.pop(s)

        # softmax over the 64 experts (free dim)
        negmax = stat.tile([ST, 1], F32, tag="negmax")
        nc.vector.reduce_max(negmax[:], scores[:], axis=mybir.AxisListType.X,
                             negate=True)
        u = probs_pool.tile([ST, E], F32, tag="u")
        sumexp = stat.tile([ST, 1], F32, tag="sumexp")
        nc.scalar.activation(u[:], scores[:], mybir.ActivationFunctionType.Exp,
                             bias=negmax[:], scale=1.0, accum_out=sumexp[:])
        rinv = stat.tile([ST, 1], F32, tag="rinv")
        nc.vector.reciprocal(rinv[:], sumexp[:])
        probs = probs_pool.tile([ST, E], F32, tag="probs")
        nc.scalar.mul(probs[:], u[:], rinv[:])
        nc.sync.dma_start(pr_d[r0:r0 + ST, :], probs[:])

        # top-8 + renormalize
        vals8 = top8.tile([ST, TOPK], F32, tag="vals8")
        nc.vector.max(vals8[:], probs[:])
        idx8 = top8.tile([ST, TOPK], U32, tag="idx8")
        nc.vector.max_index(idx8[:], vals8[:], probs[:])
        s8 = stat.tile([ST, 1], F32, tag="s8")
        nc.vector.reduce_sum(s8[:], vals8[:], axis=mybir.AxisListType.X)
        s8e = stat.tile([ST, 1], F32, tag="s8e")
        nc.vector.tensor_scalar_add(s8e[:], s8[:], 1e-9)
        r8 = stat.tile([ST, 1], F32, tag="r8")
        nc.vector.reciprocal(r8[:], s8e[:])
        vals8n = top8.tile([ST, TOPK], F32, tag="vals8n")
        nc.vector.tensor_scalar_mul(vals8n[:], vals8[:], r8[:])
        nc.sync.dma_start(pv_d[r0:r0 + ST, :], vals8n[:])
        nc.sync.dma_start(pi_d[r0:r0 + ST, :], idx8[:])

    # Chunk-interleaved software pipeline: while subtile s-1's matmuls
    # accumulate, subtile s's transposes stream in between them.
    scores_tiles = {}
    x_tiles = {}
    for s in range(NSUB + 1):
        if s < NSUB:
            x_tiles[s] = emit_load(s)
        if s >= 1:
            scores_tiles[s - 1] = sc_ps.tile([ST, E], F32, tag="sc", name=f"scores{s-1}")
        for c4 in range(NCH // 4):
            if s < NSUB:
                emit_transpose4(s, x_tiles[s], c4)
            if s >= 1:
                emit_mms(s - 1, scores_tiles[s - 1], xt_tiles[s - 1], c4)
        if s >= 1:
            xt_tiles.pop(s - 1)
            emit_softmax(s - 1)


def _build(iters=1):
    nc = bass.Bass(trn_type="TRN2", target_bir_lowering=False, debug=False)
    x_d = nc.dram_tensor("x", [NT, D], F32, kind="ExternalInput").ap()
    w_d = nc.dram_tensor("weight", [E, D], F32, kind="ExternalInput").ap()
    pv_d = nc.dram_tensor("topk_vals", [NT, TOPK], F32,
                          kind="ExternalOutput").ap()
    pi_d = nc.dram_tensor("topk_idx", [NT, TOPK], U32,
                          kind="ExternalOutput").ap()
    pr_d = nc.dram_tensor("probs", [NT, E], F32, kind="ExternalOutput").ap()

    with tile.TileContext(nc) as tc:
        from contextlib import ExitStack
        with ExitStack() as ctx:
            pools = (
                ctx.enter_context(tc.tile_pool(name="xload", bufs=4)),
                ctx.enter_context(tc.tile_pool(name="xt_sb", bufs=10)),
                ctx.enter_context(tc.tile_pool(name="xt_ps", bufs=5,
                                               space="PSUM")),
                ctx.enter_context(tc.tile_pool(name="sc_ps", bufs=3,
                                               space="PSUM")),
                ctx.enter_context(tc.tile_pool(name="wt", bufs=1)),
                ctx.enter_context(tc.tile_pool(name="w", bufs=1)),
                ctx.enter_context(tc.tile_pool(name="ident", bufs=1)),
                ctx.enter_context(tc.tile_pool(name="probs", bufs=3)),
                ctx.enter_context(tc.tile_pool(name="stat", bufs=4)),
                ctx.enter_context(tc.tile_pool(name="top8", bufs=3)),
                ctx.enter_context(tc.tile_pool(name="wp_ps", bufs=2,
                                               space="PSUM")),
            )
            for it in range(iters):
                _emit_body(nc, tc, pools, x_d, w_d, pv_d, pi_d, pr_d,
                           first=(it == 0))
    return nc


_built = {}


def _get(iters=1):
    if iters not in _built:
        _built[iters] = _build(iters)
    return _built[iters]


def run(x, weight, iters=1):
    nc = _get(iters)
    in_maps = [
        {"x": np.ascontiguousarray(x[i * NT:(i + 1) * NT]),
         "weight": np.ascontiguousarray(weight)}
        for i in range(N_CORES)
    ]
    res = run_bass_kernel_spmd(nc, in_maps, list(range(N_CORES)))
    vals = np.concatenate([res.results[i]["topk_vals"] for i in range(N_CORES)])
    idx = np.concatenate([res.results[i]["topk_idx"] for i in range(N_CORES)])
    probs = np.concatenate([res.results[i]["probs"] for i in range(N_CORES)])
    return vals, idx.astype(np.int32), probs


def kernel(x, weight):
    x = np.asarray(x, dtype=np.float32)
    weight = np.asarray(weight, dtype=np.float32)
    return run(x, weight, iters=1)


# revision 30
# speedup vs baseline: 1.1616x; 1.1591x over previous
"""MoE gate (router) kernel for Trainium2, 8 NeuronCores.

reference:
    scores = x @ weight.T          # [N, E]
    probs  = softmax(scores, -1)
    vals, idx = top_k(probs, 8)
    vals  = vals / (vals.sum(-1, keepdims) + 1e-9)
    returns (vals [N,8] f32, idx [N,8] i32, probs [N,E] f32)

Sharding: x is split along the token dim across 8 cores (1024 tokens
each); weight is replicated.  Routing is token-local, no collectives.

Layout strategy: the host hands each core its x shard pre-transposed
(xT [D, NT]) and the gate weight transposed and column-duplicated
(wT2 [D, 2E]).  The contraction dim is then on partitions for both
matmul operands straight from DRAM, so the PE runs *only* the 256
accumulating fp32 matmuls per core -- no on-chip transposes.  The
column duplication makes each fp32 half-matmul stream 128 moving
columns (~107 ns of array work per ~112 ns dispatch slot): the PE
array duty stays above the HAM clock-gate threshold, so the PE holds
2.4 GHz. With plain 64-column moving operands the array idles ~75% of
each slot and the clock stays gated at 1.2 GHz, doubling the runtime.

Per-core pipeline:
  - per chunk c: wT2 chunk DMA on one HWDGE engine, the two x
    token-half DMAs split across both (sync/scalar) -> fine-grained
    arrival, no head-of-line blocking
  - 8 concurrent PSUM accumulation groups (one bank per 128-token
    subtile): scores[s] += xT_c[:, s].T @ wT2_c  (fp32, chunk-major)
  - per subtile: softmax on [128, 64] (DVE reduce_max(neg) -> ACT exp
    with bias=-max, accum_out=sum -> DVE reciprocal -> ACT scale),
    hardware top-8 (DVE MAX8 / FIND_INDEX8), renormalize, DMA out.
"""

import numpy as np

import concourse.bass as bass
import concourse.mybir as mybir
import concourse.tile as tile
from concourse.bass_utils import run_bass_kernel_spmd

F32 = mybir.dt.float32
U32 = mybir.dt.uint32

N_CORES = 8
N, D, E, TOPK = 8192, 4096, 64, 8
NT = N // N_CORES
ST = 128
NSUB = NT // ST
CH = 128
NCH = D // CH
E2 = 2 * E               # duplicated expert columns


MAX_WAITS = 1


def _split_waits(nc, maxw=MAX_WAITS):
    """neuronxcc CoreV3 codegen rejects an instruction carrying more than
    `maxw` sem waits ("Too many sync wait commands").  Move excess waits
    onto same-engine ENGINE_NOP instructions inserted just before the
    offending instruction — the sequencer blocks on them in program order,
    which is semantically identical."""
    n = 0
    for f in nc.m.functions:
        for bb in f.blocks:
            snapshot = list(bb.instructions)
            rewritten = []
            changed = False
            for inst in snapshot:
                si = inst.sync_info
                if si is not None and si.on_wait and len(si.on_wait) > maxw:
                    waits = list(si.on_wait)
                    excess, keep = waits[:-maxw], waits[-maxw:]
                    for i in range(0, len(excess), maxw):
                        # nop() appends a well-formed nop to the *current*
                        # block; pop it off and re-home it in front of
                        # `inst` in this block.
                        nop = nc.engines[inst.engine].nop(nofuse=True).ins
                        cur_insts = nc.cur_bb.bb.instructions
                        assert cur_insts[-1].name == nop.name
                        cur_insts.pop()
                        n += 1
                        nop.sync_info = mybir.SyncInfo(
                            on_wait=excess[i:i + maxw], on_update=[])
                        rewritten.append(nop)
                    inst.sync_info = mybir.SyncInfo(
                        on_wait=keep, on_update=si.on_update)
                    changed = True
                rewritten.append(inst)
            if changed:
                bb.instructions[:] = rewritten
    return n


def _patch_tail_drain():
    """Run the wait-splitting pass after TileContext finishes emitting
    everything (tail drain included)."""

    orig = tile.TileContext._drain_and_barrier
    if getattr(orig, "_waitsplit_wrapped", False):
        return

    def _drain_and_barrier(self, tick_clock, wait_clock):
        orig(self, tick_clock, wait_clock)
        _split_waits(self.nc)

    _drain_and_barrier._waitsplit_wrapped = True
    tile.TileContext._drain_and_barrier = _drain_and_barrier


_patch_tail_drain()


def _emit_body(nc, tc, pools, xt_d, wt_d, pv_d, pi_d, pr_d, first):
    (xtload, wt_pool, probs_pool, stat, top8, sc_ps) = pools

    # wT2 [D, 2E] -> SBUF [128, NCH*2E]; chunk c at cols [E2*c, E2*(c+1))
    wt_sb = wt_pool.tile([128, NCH * E2], F32, tag="wt", name="wt_all")
    for c in range(NCH):
        nc.scalar.dma_start(wt_sb[:, c * E2:(c + 1) * E2],
                            wt_d[c * CH:(c + 1) * CH, :])

    scores_tiles = [
        sc_ps.tile([ST, E2], F32, tag="sc", name=f"scores{s}")
        for s in range(NSUB)
    ]

    chunk_tiles = []
    for c in range(NCH):
        xt_ch = xtload.tile([CH, NT], F32, tag="xt", name=f"xt{c}")
        eng = nc.sync if c % 2 == 0 else nc.scalar
        eng.dma_start(xt_ch[:], xt_d[c * CH:(c + 1) * CH, :])
        chunk_tiles.append(xt_ch)

    # chunk-major sweep: 8 concurrent accumulation groups
    for c in range(NCH):
        for s in range(NSUB):
            nc.tensor.matmul(scores_tiles[s][:],
                             chunk_tiles[c][:, s * ST:(s + 1) * ST],
                             wt_sb[:, c * E2:(c + 1) * E2],
                             start=(c == 0), stop=(c == NCH - 1))

    for s in range(NSUB):
        r0 = s * ST
        scores = scores_tiles[s][:, :E]

        negmax = stat.tile([ST, 1], F32, tag="negmax")
        nc.vector.reduce_max(negmax[:], scores, axis=mybir.AxisListType.X,
                             negate=True)
        u = probs_pool.tile([ST, E], F32, tag="u")
        sumexp = stat.tile([ST, 1], F32, tag="sumexp")
        nc.scalar.activation(u[:], scores, mybir.ActivationFunctionType.Exp,
                             bias=negmax[:], scale=1.0, accum_out=sumexp[:])
        rinv = stat.tile([ST, 1], F32, tag="rinv")
        nc.vector.reciprocal(rinv[:], sumexp[:])
        probs = probs_pool.tile([ST, E], F32, tag="probs")
        nc.scalar.mul(probs[:], u[:], rinv[:])
        nc.sync.dma_start(pr_d[r0:r0 + ST, :], probs[:])

        vals8 = top8.tile([ST, TOPK], F32, tag="vals8")
        nc.vector.max(vals8[:], probs[:])
        idx8 = top8.tile([ST, TOPK], U32, tag="idx8")
        nc.vector.max_index(idx8[:], vals8[:], probs[:])
        s8 = stat.tile([ST, 1], F32, tag="s8")
        nc.vector.reduce_sum(s8[:], vals8[:], axis=mybir.AxisListType.X)
        s8e = stat.tile([ST, 1], F32, tag="s8e")
        nc.vector.tensor_scalar_add(s8e[:], s8[:], 1e-9)
        r8 = stat.tile([ST, 1], F32, tag="r8")
        nc.vector.reciprocal(r8[:], s8e[:])
        vals8n = top8.tile([ST, TOPK], F32, tag="vals8n")
        nc.vector.tensor_scalar_mul(vals8n[:], vals8[:], r8[:])
        nc.sync.dma_start(pv_d[r0:r0 + ST, :], vals8n[:])
        nc.sync.dma_start(pi_d[r0:r0 + ST, :], idx8[:])


def _build(iters=1):
    nc = bass.Bass(trn_type="TRN2", target_bir_lowering=False, debug=False)
    xt_d = nc.dram_tensor("xt", [D, NT], F32, kind="ExternalInput").ap()
    wt_d = nc.dram_tensor("wt2", [D, E2], F32, kind="ExternalInput").ap()
    pv_d = nc.dram_tensor("topk_vals", [NT, TOPK], F32,
                          kind="ExternalOutput").ap()
    pi_d = nc.dram_tensor("topk_idx", [NT, TOPK], U32,
                          kind="ExternalOutput").ap()
    pr_d = nc.dram_tensor("probs", [NT, E], F32, kind="ExternalOutput").ap()

    with tile.TileContext(nc) as tc:
        from contextlib import ExitStack
        with ExitStack() as ctx:
            pools = (
                ctx.enter_context(tc.tile_pool(name="xtload", bufs=6)),
                ctx.enter_context(tc.tile_pool(name="wt", bufs=1)),
                ctx.enter_context(tc.tile_pool(name="probs", bufs=3)),
                ctx.enter_context(tc.tile_pool(name="stat", bufs=4)),
                ctx.enter_context(tc.tile_pool(name="top8", bufs=3)),
                ctx.enter_context(tc.tile_pool(name="sc_ps", bufs=NSUB,
                                               space="PSUM")),
            )
            for it in range(iters):
                _emit_body(nc, tc, pools, xt_d, wt_d, pv_d, pi_d, pr_d,
                           first=(it == 0))
    return nc


_built = {}


def _get(iters=1):
    if iters not in _built:
        _built[iters] = _build(iters)
    return _built[iters]


def make_in_maps(x, weight):
    wt2 = np.ascontiguousarray(np.tile(weight.T, (1, 2)))   # [D, 2E]
    xt_full = np.asarray(x).T
    return [
        {"xt": np.ascontiguousarray(xt_full[:, i * NT:(i + 1) * NT]),
         "wt2": wt2}
        for i in range(N_CORES)
    ]


def run(x, weight, iters=1, trace=False, tmpdir=None):
    nc = _get(iters)
    res = run_bass_kernel_spmd(nc, make_in_maps(x, weight),
                               list(range(N_CORES)), trace=trace,
                               tmpdir=tmpdir)
    vals = np.concatenate([res.results[i]["topk_vals"] for i in range(N_CORES)])
    idx = np.concatenate([res.results[i]["topk_idx"] for i in range(N_CORES)])
    probs = np.concatenate([res.results[i]["probs"] for i in range(N_CORES)])
    return (vals, idx.astype(np.int32), probs), res


def kernel(x, weight):
    x = np.asarray(x, dtype=np.float32)
    weight = np.asarray(weight, dtype=np.float32)
    out, _ = run(x, weight, iters=1)
    return out


# revision 32
# speedup vs baseline: 1.1754x; 1.0118x over previous
"""MoE gate (router) kernel for Trainium2, 8 NeuronCores.

reference:
    scores = x @ weight.T          # [N, E]
    probs  = softmax(scores, -1)
    vals, idx = top_k(probs, 8)
    vals  = vals / (vals.sum(-1, keepdims) + 1e-9)
    returns (vals [N,8] f32, idx [N,8] i32, probs [N,E] f32)

Sharding: x is split along the token dim across 8 cores (1024 tokens
each); weight is replicated.  Routing is token-local, no collectives.

Layout strategy: the host hands each core its x shard pre-transposed
(xT [D, NT]) and the gate weight transposed and column-duplicated
(wT2 [D, 2E]).  The contraction dim is then on partitions for both
matmul operands straight from DRAM, so the PE runs *only* the 256
accumulating fp32 matmuls per core -- no on-chip transposes.  The
column duplication makes each fp32 half-matmul stream 128 moving
columns (~107 ns of array work per ~112 ns dispatch slot): the PE
array duty stays above the HAM clock-gate threshold, so the PE holds
2.4 GHz. With plain 64-column moving operands the array idles ~75% of
each slot and the clock stays gated at 1.2 GHz, doubling the runtime.

Per-core pipeline:
  - per chunk c: wT2 chunk DMA on one HWDGE engine, the two x
    token-half DMAs split across both (sync/scalar) -> fine-grained
    arrival, no head-of-line blocking
  - 8 concurrent PSUM accumulation groups (one bank per 128-token
    subtile): scores[s] += xT_c[:, s].T @ wT2_c  (fp32, chunk-major)
  - per subtile: softmax on [128, 64] (DVE reduce_max(neg) -> ACT exp
    with bias=-max, accum_out=sum -> DVE reciprocal -> ACT scale),
    hardware top-8 (DVE MAX8 / FIND_INDEX8), renormalize, DMA out.
"""

import numpy as np

import concourse.bass as bass
import concourse.mybir as mybir
import concourse.tile as tile
from concourse.bass_utils import run_bass_kernel_spmd

F32 = mybir.dt.float32
U32 = mybir.dt.uint32

N_CORES = 8
N, D, E, TOPK = 8192, 4096, 64, 8
NT = N // N_CORES
ST = 128
NSUB = NT // ST
CH = 128
NCH = D // CH
E2 = 2 * E               # duplicated expert columns


MAX_WAITS = 1


def _split_waits(nc, maxw=MAX_WAITS):
    """neuronxcc CoreV3 codegen rejects an instruction carrying more than
    `maxw` sem waits ("Too many sync wait commands").  Move excess waits
    onto same-engine ENGINE_NOP instructions inserted just before the
    offending instruction — the sequencer blocks on them in program order,
    which is semantically identical."""
    n = 0
    for f in nc.m.functions:
        for bb in f.blocks:
            snapshot = list(bb.instructions)
            rewritten = []
            changed = False
            for inst in snapshot:
                si = inst.sync_info
                if si is not None and si.on_wait and len(si.on_wait) > maxw:
                    waits = list(si.on_wait)
                    excess, keep = waits[:-maxw], waits[-maxw:]
                    for i in range(0, len(excess), maxw):
                        # nop() appends a well-formed nop to the *current*
                        # block; pop it off and re-home it in front of
                        # `inst` in this block.
                        nop = nc.engines[inst.engine].nop(nofuse=True).ins
                        cur_insts = nc.cur_bb.bb.instructions
                        assert cur_insts[-1].name == nop.name
                        cur_insts.pop()
                        n += 1
                        nop.sync_info = mybir.SyncInfo(
                            on_wait=excess[i:i + maxw], on_update=[])
                        rewritten.append(nop)
                    inst.sync_info = mybir.SyncInfo(
                        on_wait=keep, on_update=si.on_update)
                    changed = True
                rewritten.append(inst)
            if changed:
                bb.instructions[:] = rewritten
    return n


def _patch_tail_drain():
    """Run the wait-splitting pass after TileContext finishes emitting
    everything (tail drain included)."""

    orig = tile.TileContext._drain_and_barrier
    if getattr(orig, "_waitsplit_wrapped", False):
        return

    def _drain_and_barrier(self, tick_clock, wait_clock):
        orig(self, tick_clock, wait_clock)
        _split_waits(self.nc)

    _drain_and_barrier._waitsplit_wrapped = True
    tile.TileContext._drain_and_barrier = _drain_and_barrier


_patch_tail_drain()


def _emit_body(nc, tc, pools, xt_d, wt_d, pv_d, pi_d, pr_d, first):
    (xtload, wt_pool, probs_pool, stat, top8, sc_ps) = pools

    # wT2 [D, 2E] -> SBUF [128, NCH*2E]; chunk c at cols [E2*c, E2*(c+1))
    wt_sb = wt_pool.tile([128, NCH * E2], F32, tag="wt", name="wt_all")
    for c in range(NCH):
        nc.scalar.dma_start(wt_sb[:, c * E2:(c + 1) * E2],
                            wt_d[c * CH:(c + 1) * CH, :])

    scores_tiles = [
        sc_ps.tile([ST, E2], F32, tag="sc", name=f"scores{s}")
        for s in range(NSUB)
    ]

    chunk_tiles = []
    for c in range(NCH):
        xt_ch = xtload.tile([CH, NT], F32, tag="xt", name=f"xt{c}")
        eng = nc.sync if c % 2 == 0 else nc.scalar
        eng.dma_start(xt_ch[:], xt_d[c * CH:(c + 1) * CH, :])
        chunk_tiles.append(xt_ch)

    # chunk-major sweep: 8 concurrent accumulation groups
    for c in range(NCH):
        for s in range(NSUB):
            nc.tensor.matmul(scores_tiles[s][:],
                             chunk_tiles[c][:, s * ST:(s + 1) * ST],
                             wt_sb[:, c * E2:(c + 1) * E2],
                             start=(c == 0), stop=(c == NCH - 1))

    for s in range(NSUB):
        r0 = s * ST
        scores = scores_tiles[s][:, :E]

        negmax = stat.tile([ST, 1], F32, tag="negmax")
        nc.vector.reduce_max(negmax[:], scores, axis=mybir.AxisListType.X,
                             negate=True)
        u = probs_pool.tile([ST, E], F32, tag="u")
        sumexp = stat.tile([ST, 1], F32, tag="sumexp")
        nc.scalar.activation(u[:], scores, mybir.ActivationFunctionType.Exp,
                             bias=negmax[:], scale=1.0, accum_out=sumexp[:])
        rinv = stat.tile([ST, 1], F32, tag="rinv")
        nc.vector.reciprocal(rinv[:], sumexp[:])
        probs = probs_pool.tile([ST, E], F32, tag="probs")
        nc.scalar.mul(probs[:], u[:], rinv[:])
        nc.sync.dma_start(pr_d[r0:r0 + ST, :], probs[:])

        vals8 = top8.tile([ST, TOPK], F32, tag="vals8")
        nc.vector.max(vals8[:], probs[:])
        idx8 = top8.tile([ST, TOPK], U32, tag="idx8")
        nc.vector.max_index(idx8[:], vals8[:], probs[:])
        s8 = stat.tile([ST, 1], F32, tag="s8")
        nc.vector.reduce_sum(s8[:], vals8[:], axis=mybir.AxisListType.X)
        s8e = stat.tile([ST, 1], F32, tag="s8e")
        nc.vector.tensor_scalar_add(s8e[:], s8[:], 1e-9)
        r8 = stat.tile([ST, 1], F32, tag="r8")
        nc.vector.reciprocal(r8[:], s8e[:])
        vals8n = top8.tile([ST, TOPK], F32, tag="vals8n")
        nc.vector.tensor_scalar_mul(vals8n[:], vals8[:], r8[:])
        nc.sync.dma_start(pv_d[r0:r0 + ST, :], vals8n[:])
        nc.sync.dma_start(pi_d[r0:r0 + ST, :], idx8[:])


def _build(iters=1):
    nc = bass.Bass(trn_type="TRN2", target_bir_lowering=False, debug=False)
    xt_d = nc.dram_tensor("xt", [D, NT], F32, kind="ExternalInput").ap()
    wt_d = nc.dram_tensor("wt2", [D, E2], F32, kind="ExternalInput").ap()
    pv_d = nc.dram_tensor("topk_vals", [NT, TOPK], F32,
                          kind="ExternalOutput").ap()
    pi_d = nc.dram_tensor("topk_idx", [NT, TOPK], U32,
                          kind="ExternalOutput").ap()
    pr_d = nc.dram_tensor("probs", [NT, E], F32, kind="ExternalOutput").ap()

    with tile.TileContext(nc) as tc:
        from contextlib import ExitStack
        with ExitStack() as ctx:
            pools = (
                ctx.enter_context(tc.tile_pool(name="xtload", bufs=6)),
                ctx.enter_context(tc.tile_pool(name="wt", bufs=1)),
                ctx.enter_context(tc.tile_pool(name="probs", bufs=3)),
                ctx.enter_context(tc.tile_pool(name="stat", bufs=4)),
                ctx.enter_context(tc.tile_pool(name="top8", bufs=3)),
                ctx.enter_context(tc.tile_pool(name="sc_ps", bufs=NSUB,
                                               space="PSUM")),
            )
            for it in range(iters):
                _emit_body(nc, tc, pools, xt_d, wt_d, pv_d, pi_d, pr_d,
                           first=(it == 0))
    return nc


_built = {}


def _get(iters=1):
    if iters not in _built:
        _built[iters] = _build(iters)
    return _built[iters]


def make_in_maps(x, weight):
    wt2 = np.ascontiguousarray(np.tile(weight.T, (1, 2)))   # [D, 2E]
    xt_full = np.asarray(x).T
    return [
        {"xt": np.ascontiguousarray(xt_full[:, i * NT:(i + 1) * NT]),
         "wt2": wt2}
        for i in range(N_CORES)
    ]


def run(x, weight, iters=1, trace=False, tmpdir=None):
    nc = _get(iters)
    res = run_bass_kernel_spmd(nc, make_in_maps(x, weight),
                               list(range(N_CORES)), trace=trace,
                               tmpdir=tmpdir)
    vals = np.concatenate([res.results[i]["topk_vals"] for i in range(N_CORES)])
    idx = np.concatenate([res.results[i]["topk_idx"] for i in range(N_CORES)])
    probs = np.concatenate([res.results[i]["probs"] for i in range(N_CORES)])
    return (vals, idx.astype(np.int32), probs), res


def kernel(x, weight):
    x = np.asarray(x, dtype=np.float32)
    weight = np.asarray(weight, dtype=np.float32)
    out, _ = run(x, weight, iters=1)
    return out
